# revision 5
# baseline (speedup 1.0000x reference)
"""Bass/Trainium2 kernel for nn_BayesMultiheadAttention (B=4,T=2048,D=1024,H=8).

Sharding: tensor-parallel over heads. Core c computes head c (QKV proj +
causal attention) for all 4 batches; a per-batch AllToAll redistributes
per-head outputs into per-token-slice outputs (pipelined against the next
batch's compute); each core then does the multiplicative reduce over heads
and its slice of out_proj.

x and the QKV weights are converted to bf16 on the host and DMA'd straight
into SBUF (no on-chip rounding passes); all projection/attention matmuls
run in bf16 (1 cycle/row at any free size), accumulating fp32 in PSUM.
Scores, softmax normalization, the AllToAll payload, the head product and
out_proj stay fp32/f32r. Softmax denominators are accumulated in PSUM by
per-tile ones-matmuls. Dropout masks and the 1/sqrt(HD) scale are folded
into per-(core,batch) weight copies on the host. Softmax skips
max-subtraction (scores are O(5), exp cannot overflow).
"""
import numpy as np

import concourse.bacc as bacc
import concourse.mybir as mybir
import concourse.tile as tile
from concourse.bass_utils import run_bass_kernel_spmd

B, T, D, H = 4, 2048, 1024, 8
HD = 128          # head dim
P = 128           # partitions
NC = 8            # cores
TQ = 512          # qt chunk width
NKD = D // P      # 8 contraction tiles
NTT = T // P      # 16 token tiles per batch
NQC = T // TQ     # 4 qt chunks per batch
TS = T // NC      # 256: per-core token slice of one batch
TOK_SLICE = B * TS  # 1024 tokens per core in the tail

dt = mybir.dt
F32 = dt.float32
F32R = dt.float32r
F16 = dt.float16

_PROGRAM = {}


def build_program(nreps=1):
    global _PROGRAM
    if nreps in _PROGRAM:
        return _PROGRAM[nreps]
    nc = bacc.Bacc("TRN2", target_bir_lowering=False, debug=False,
                   num_devices=NC)

    xT_d = nc.dram_tensor("xT", [B, D, T], F16, kind="ExternalInput")
    wq_d = nc.dram_tensor("wq", [B, NKD, P, HD], F16, kind="ExternalInput")
    wk_d = nc.dram_tensor("wk", [B, NKD, P, HD], F16, kind="ExternalInput")
    wv_d = nc.dram_tensor("wv", [B, NKD, P, HD], F16, kind="ExternalInput")
    wo_d = nc.dram_tensor("wo", [HD, D], F32, kind="ExternalInput")
    cm_d = nc.dram_tensor("cm", [4, P, TQ], F16, kind="ExternalInput")
    eye_d = nc.dram_tensor("eye", [P, P], F16, kind="ExternalInput")
    y_d = nc.dram_tensor("y", [TOK_SLICE, D], F32, kind="ExternalOutput")

    rg = [list(range(NC))]
    Exp = mybir.ActivationFunctionType.Exp

    from contextlib import ExitStack
    with tile.TileContext(nc) as tc, ExitStack() as ctx:
        ec = ctx.enter_context
        constp = ec(tc.tile_pool(name="const", bufs=1))
        xrp = ec(tc.tile_pool(name="xr", bufs=1))
        wrp = ec(tc.tile_pool(name="wr", bufs=1))
        qkvp = ec(tc.tile_pool(name="qkv", bufs=1))
        eop = ec(tc.tile_pool(name="eo", bufs=6))
        scp = ec(tc.tile_pool(name="sc", bufs=3))
        outbp = ec(tc.tile_pool(name="outb", bufs=1))
        tailp = ec(tc.tile_pool(name="tail", bufs=2))
        hpp = ec(tc.tile_pool(name="hp", bufs=2))
        ysbp = ec(tc.tile_pool(name="ysb", bufs=2))
        psA = ec(tc.tile_pool(name="psA", bufs=2, space="PSUM"))
        psS = ec(tc.tile_pool(name="psS", bufs=2, space="PSUM"))
        psO = ec(tc.tile_pool(name="psO", bufs=2, space="PSUM"))
        psD = ec(tc.tile_pool(name="psD", bufs=2, space="PSUM"))
        dram = ec(tc.tile_pool(name="dram", bufs=1, space="DRAM"))
        a2a_in = [dram.tile([NC, P, TS], F16, name=f"a2a_in{b}",
                            tag=f"a2a_in{b}") for b in range(B)]
        a2a_out = [dram.tile([NC, P, TS], F16, name=f"a2a_out{b}",
                             tag=f"a2a_out{b}") for b in range(B)]

        ones_b = constp.tile([P, P], F16, name="ones_b", tag="ones_b")
        nc.vector.memset(ones_b[:], 1.0)

        eye_b = constp.tile([P, P], F16, name="eye_b", tag="eye_b")
        nc.sync.dma_start(eye_b[:], eye_d.ap())

        cm_sb = constp.tile([P, 4 * TQ], F16, name="cm_sb", tag="cm_sb")
        nc.sync.dma_start(cm_sb[:], cm_d.ap().rearrange("j p q -> p j q"))

        wo_st = constp.tile([P, D], F32, name="wo_st", tag="wo_st")
        nc.sync.dma_start(wo_st[:], wo_d.ap())
        wor = constp.tile([P, D], F32R, name="wor", tag="wor")
        nc.vector.tensor_copy(wor[:], wo_st[:])

        prodr = tailp.tile([P, TOK_SLICE], F32R, name="prodr",
                           tag="prodr", bufs=1)

        tail_pr = {}

        def emit_tail_head(b):
            """Start consuming A2A(b): head product chain on Pool."""
            hp = hpp.tile([P, NC * TS], F16, name="hp", tag="hp")
            nc.gpsimd.dma_start(
                hp[:], a2a_out[b].rearrange("r p t -> p r t"))
            pr = tailp.tile([P, TS], F32, name="pr", tag="pr")
            nc.gpsimd.tensor_mul(pr[:], hp[:, 0:TS], hp[:, TS:2 * TS])
            for r in range(2, NC - 1):
                nc.gpsimd.tensor_mul(
                    pr[:], pr[:], hp[:, r * TS:(r + 1) * TS])
            tail_pr[b] = (pr, hp)

        def emit_tail_tail(b):
            """Finish A2A(b): final product multiply + out_proj slice."""
            pr, hp = tail_pr.pop(b)
            nc.vector.tensor_mul(
                prodr[:, b * TS:(b + 1) * TS], pr[:],
                hp[:, (NC - 1) * TS:NC * TS])
            for ttl in range(TS // P):
                tt = b * (TS // P) + ttl
                ysb = ysbp.tile([P, D], F32, name="ysb", tag="ysb")
                for nn in range(D // TQ):
                    accy = psA.tile([P, TQ], F32, name="accy",
                                    tag="mmacc")
                    nc.tensor.matmul(
                        accy[:],
                        prodr[:, tt * P:(tt + 1) * P],
                        wor[:, nn * TQ:(nn + 1) * TQ],
                        start=True, stop=True)
                    nc.vector.tensor_copy(
                        ysb[:, nn * TQ:(nn + 1) * TQ], accy[:])
                nc.sync.dma_start(y_d.ap()[tt * P:(tt + 1) * P, :],
                                  ysb[:])

        staged = {}
        pending = {}

        def make_load_steps(b):
            """Closures that DMA batch b's x and weights into SBUF (bf16)."""
            st = {"wr": {}}
            staged[b] = st

            def mk_x(kd):
                def f():
                    if "xr" not in st:
                        st["xr"] = xrp.tile([P, NKD * T], F16,
                                            name="xr", tag="xr")
                    eng = nc.sync if kd % 2 == 0 else nc.gpsimd
                    eng.dma_start(st["xr"][:, kd * T:(kd + 1) * T],
                                  xT_d.ap()[b, kd * P:(kd + 1) * P, :])
                return f

            def mk_w(nm, wd):
                def f():
                    wt = wrp.tile([P, NKD * HD], F16, name=f"wr_{nm}",
                                  tag=f"wr_{nm}")
                    half = NKD // 2 * HD
                    rr = wd.ap()[b].rearrange("kd p m -> p kd m")
                    nc.sync.dma_start(wt[:, 0:half], rr[:, 0:NKD // 2])
                    nc.gpsimd.dma_start(wt[:, half:], rr[:, NKD // 2:])
                    st["wr"][nm] = wt
                return f

            wsteps = [mk_w(nm, wd)
                      for nm, wd in (("v", wv_d), ("q", wq_d),
                                     ("k", wk_d))]
            xsteps = [mk_x(kd) for kd in range(NKD)]
            steps = [wsteps[0], xsteps[0], xsteps[1], wsteps[1],
                     xsteps[2], xsteps[3], wsteps[2]] + xsteps[4:]
            return steps

        def drain_pending(b, n=None):
            steps = pending.get(b, [])
            k = len(steps) if n is None else min(n, len(steps))
            for f in steps[:k]:
                f()
            pending[b] = steps[k:]

        for rep in range(nreps):
            first = rep == 0
            for b in range(B):
                if b == 0:
                    if first:
                        pending[0] = make_load_steps(0)
                    drain_pending(0)
                else:
                    drain_pending(b)
                st = staged[b]
                xr = st["xr"]

                # ---- projections: v first, then q, k
                qkt = {}
                v_sb = None
                for nm in ("v", "q", "k"):
                    wt = st["wr"][nm]

                    dest = qkvp.tile([P, T], F16, name=f"{nm}T",
                                     tag=f"{nm}T")
                    if b == 0 and first and nm == "v":
                        # kd-outer: start PE as soon as the first x tile
                        # lands; 4 chunk accumulators across spare banks
                        accs4 = [
                            (psS if i < 2 else psO).tile(
                                [P, TQ], F32, name=f"pacc{i}",
                                tag="accs" if i < 2 else "acco")
                            for i in range(NQC)]
                        for kd in range(NKD):
                            for qc in range(NQC):
                                nc.tensor.matmul(
                                    accs4[qc][:],
                                    wt[:, kd * HD:(kd + 1) * HD],
                                    xr[:, kd * T + qc * TQ: kd * T + (qc + 1) * TQ],
                                    start=(kd == 0), stop=(kd == NKD - 1))
                        for qc in range(NQC):
                            nc.vector.tensor_copy(
                                dest[:, qc * TQ:(qc + 1) * TQ], accs4[qc][:])
                    else:
                        for qc in range(NQC):
                            acc = psA.tile([P, TQ], F32, name="acc",
                                           tag="mmacc")
                            for kd in range(NKD):
                                nc.tensor.matmul(
                                    acc[:],
                                    wt[:, kd * HD:(kd + 1) * HD],
                                    xr[:, kd * T + qc * TQ: kd * T + (qc + 1) * TQ],
                                    start=(kd == 0), stop=(kd == NKD - 1))
                            nc.vector.tensor_copy(
                                dest[:, qc * TQ:(qc + 1) * TQ], acc[:])
                    qkt[nm] = dest

                    if nm == "v":
                        # flip V to (tok parts, hd free) via PE transposes
                        v_sb = qkvp.tile([P, NTT * HD], F16, name="vS",
                                         tag="vS")
                        for tt in range(NTT):
                            vtp = psA.tile([P, P], F16, name="vtp",
                                           tag="mmacc")
                            nc.tensor.transpose(
                                vtp[:], dest[:, tt * P:(tt + 1) * P],
                                eye_b[:])
                            nc.vector.tensor_copy(
                                v_sb[:, tt * HD:(tt + 1) * HD], vtp[:])

                # ---- causal attention, scoresT layout ----
                out_b = outbp.tile([P, T], F16, name="out_b", tag="out_b")
                for qc in range(NQC):
                    nkt = 4 * (qc + 1)
                    acco = psO.tile([P, TQ], F32, name="acco", tag="acco")
                    denb = psD.tile([P, TQ], F32, name="denb", tag="denb")
                    for kt in range(nkt):
                        accs = psS.tile([P, TQ], F32, name="accs",
                                        tag="accs")
                        nc.tensor.matmul(
                            accs[:],
                            qkt["k"][:, kt * P:(kt + 1) * P],
                            qkt["q"][:, qc * TQ:(qc + 1) * TQ],
                            start=True, stop=True)
                        e = eop.tile([P, TQ], F16, name="e", tag="e")
                        nc.scalar.activation(e[:], accs[:], Exp)
                        j = kt - 4 * qc
                        if j >= 0:  # diagonal-crossing tile: zero invalid
                            nc.vector.tensor_mul(
                                e[:], e[:], cm_sb[:, j * TQ:(j + 1) * TQ])
                        nc.tensor.matmul(
                            acco[:],
                            v_sb[:, kt * HD:(kt + 1) * HD],
                            e[:],
                            start=(kt == 0), stop=(kt == nkt - 1))
                        nc.tensor.matmul(
                            denb[:], ones_b[:], e[:],
                            start=(kt == 0), stop=(kt == nkt - 1))
                    recb = scp.tile([P, TQ], F32, name="recb", tag="recb")
                    nc.vector.reciprocal_approx_fast(recb[:], denb[:])
                    nc.vector.tensor_mul(
                        out_b[:, qc * TQ:(qc + 1) * TQ], acco[:], recb[:])

                    if qc == 1 and not (b == 0 and first):
                        pb = b - 1 if b > 0 else B - 1
                        emit_tail_tail(pb)
                    if b + 1 < B:
                        if qc == 0:
                            pending[b + 1] = make_load_steps(b + 1)
                        drain_pending(b + 1, 4)
                    elif rep + 1 < nreps:
                        if qc == 0:
                            pending[0] = make_load_steps(0)
                        drain_pending(0, 4)

                # ---- ship normalized head-output
                for j in range(NC):
                    nc.sync.dma_start(a2a_in[b][j],
                                      out_b[:, j * TS:(j + 1) * TS])
                nc.gpsimd.collective_compute(
                    "AllToAll", mybir.AluOpType.bypass,
                    replica_groups=rg,
                    ins=[a2a_in[b].opt()], outs=[a2a_out[b].opt()])
                emit_tail_head(b)

        emit_tail_tail(B - 1)

    nc.compile()
    _PROGRAM[nreps] = nc
    return nc


def make_in_maps(x, Wq, Wk, Wv, Wout, q_mask, k_mask, v_mask):
    x = np.asarray(x, np.float32)
    xT = np.ascontiguousarray(x.transpose(0, 2, 1)).astype(np.float16)  # (B, D, T)
    wo = np.ascontiguousarray(np.asarray(Wout, np.float32).T)  # (HD, D)

    cm = np.zeros((4, P, TQ), np.float32)
    for j in range(4):
        for i in range(P):
            cm[j, i, j * P + i:] = 1.0
    cm = cm.astype(np.float16)
    eye = np.eye(P, dtype=np.float32).astype(np.float16)

    s = np.float32(1.0 / np.sqrt(HD))
    q_mask = np.asarray(q_mask, np.float32)
    k_mask = np.asarray(k_mask, np.float32)
    v_mask = np.asarray(v_mask, np.float32)
    Wq = np.asarray(Wq, np.float32)
    Wk = np.asarray(Wk, np.float32)
    Wv = np.asarray(Wv, np.float32)

    in_maps = []
    for c in range(NC):
        def pack(W, m, scale):
            out = np.empty((B, NKD, P, HD), np.float32)
            Wh = W[c * HD:(c + 1) * HD, :]                  # (HD, D)
            for b in range(B):
                Wp = (Wh * (m[b, c, 0, :, None] * scale)).T  # (D, HD)
                out[b] = Wp.reshape(NKD, P, HD)
            return out.astype(np.float16)
        in_maps.append({
            "xT": xT,
            "wq": pack(Wq, q_mask, s),
            "wk": pack(Wk, k_mask, np.float32(1.0)),
            "wv": pack(Wv, v_mask, np.float32(1.0)),
            "wo": wo,
            "cm": cm,
            "eye": eye,
        })
    return in_maps


def kernel(x, Wq, Wk, Wv, Wout, q_mask, k_mask, v_mask, mask=None):
    nc = build_program()
    in_maps = make_in_maps(x, Wq, Wk, Wv, Wout, q_mask, k_mask, v_mask)
    res = run_bass_kernel_spmd(nc, in_maps, core_ids=list(range(NC))).results
    # core c's y rows are ordered (b, local-token); its tokens are
    # [c*TS, (c+1)*TS) of every batch
    out = np.empty((B, T, D), np.float32)
    for c in range(NC):
        yc = res[c]["y"].reshape(B, TS, D)
        out[:, c * TS:(c + 1) * TS, :] = yc
    return out


# revision 6
# speedup vs baseline: 2047918.8862x; 2047918.8862x over previous
"""Bass/Trainium2 kernel for nn_BayesMultiheadAttention (B=4,T=2048,D=1024,H=8).

Sharding: tensor-parallel over heads. Core c computes head c (QKV proj +
causal attention) for all 4 batches; a per-batch AllToAll redistributes
per-head outputs into per-token-slice outputs (pipelined against the next
batch's compute); each core then does the multiplicative reduce over heads
and its slice of out_proj.

x and the QKV weights are fp16 on the host and DMA'd straight into SBUF
(no on-chip rounding passes); projection/attention matmuls run in fp16
(1 cycle/row at any free size), accumulating fp32 in PSUM. Scores PSUM,
softmax normalization, head product and out_proj stay fp32/f32r; the
AllToAll payload is fp16.

Attention runs on [128,1024] score PAIR tiles (2 PSUM banks each, 2 bufs
= 4 banks): one exp ACTIVATE covers two kt tiles, and emission is skewed
(scores of pair p+1 issue before AV/denb of pair p) so ACT latency hides
behind PE work. Softmax denominators accumulate in PSUM via per-tile
ones-matmuls. V transposes are interleaved into the Q projection chunks
to keep PE fed while DVE evicts. Dropout masks and the 1/sqrt(HD) scale
are folded into per-(core,batch) weight copies on the host. Softmax skips
max-subtraction (scores are small; exp cannot overflow).
"""
import numpy as np

import concourse.bacc as bacc
import concourse.mybir as mybir
import concourse.tile as tile
from concourse.bass_utils import run_bass_kernel_spmd

B, T, D, H = 4, 2048, 1024, 8
HD = 128          # head dim
P = 128           # partitions
NC = 8            # cores
TQ = 512          # qt chunk width
NKD = D // P      # 8 contraction tiles
NTT = T // P      # 16 token tiles per batch
NQC = T // TQ     # 4 qt chunks per batch
TS = T // NC      # 256: per-core token slice of one batch
TOK_SLICE = B * TS  # 1024 tokens per core in the tail

dt = mybir.dt
F32 = dt.float32
F32R = dt.float32r
F16 = dt.float16

_PROGRAM = {}


def build_program(nreps=1):
    global _PROGRAM
    if nreps in _PROGRAM:
        return _PROGRAM[nreps]
    nc = bacc.Bacc("TRN2", target_bir_lowering=False, debug=False,
                   num_devices=NC)

    xT_d = nc.dram_tensor("xT", [B, D, T], F16, kind="ExternalInput")
    wq_d = nc.dram_tensor("wq", [B, NKD, P, HD], F16, kind="ExternalInput")
    wk_d = nc.dram_tensor("wk", [B, NKD, P, HD], F16, kind="ExternalInput")
    wv_d = nc.dram_tensor("wv", [B, NKD, P, HD], F16, kind="ExternalInput")
    wo_d = nc.dram_tensor("wo", [HD, D], F32, kind="ExternalInput")
    cm_d = nc.dram_tensor("cm", [4, P, TQ], F16, kind="ExternalInput")
    eye_d = nc.dram_tensor("eye", [P, P], F16, kind="ExternalInput")
    y_d = nc.dram_tensor("y", [TOK_SLICE, D], F32, kind="ExternalOutput")

    rg = [list(range(NC))]
    Exp = mybir.ActivationFunctionType.Exp

    from contextlib import ExitStack
    with tile.TileContext(nc) as tc, ExitStack() as ctx:
        ec = ctx.enter_context
        constp = ec(tc.tile_pool(name="const", bufs=1))
        xrp = ec(tc.tile_pool(name="xr", bufs=1))
        wrp = ec(tc.tile_pool(name="wr", bufs=1))
        qkvp = ec(tc.tile_pool(name="qkv", bufs=1))
        eop = ec(tc.tile_pool(name="eo", bufs=4))
        scp = ec(tc.tile_pool(name="sc", bufs=3))
        outbp = ec(tc.tile_pool(name="outb", bufs=1))
        tailp = ec(tc.tile_pool(name="tail", bufs=2))
        hpp = ec(tc.tile_pool(name="hp", bufs=2))
        ysbp = ec(tc.tile_pool(name="ysb", bufs=2))
        psA = ec(tc.tile_pool(name="psA", bufs=2, space="PSUM"))
        psS = ec(tc.tile_pool(name="psS", bufs=2, space="PSUM"))
        psO = ec(tc.tile_pool(name="psO", bufs=1, space="PSUM"))
        psD = ec(tc.tile_pool(name="psD", bufs=1, space="PSUM"))
        dram = ec(tc.tile_pool(name="dram", bufs=1, space="DRAM"))
        a2a_in = [dram.tile([NC, P, TS], F16, name=f"a2a_in{b}",
                            tag=f"a2a_in{b}") for b in range(B)]
        a2a_out = [dram.tile([NC, P, TS], F16, name=f"a2a_out{b}",
                             tag=f"a2a_out{b}") for b in range(B)]

        ones_b = constp.tile([P, P], F16, name="ones_b", tag="ones_b")
        nc.vector.memset(ones_b[:], 1.0)

        eye_b = constp.tile([P, P], F16, name="eye_b", tag="eye_b")
        nc.sync.dma_start(eye_b[:], eye_d.ap())

        cm_sb = constp.tile([P, 4 * TQ], F16, name="cm_sb", tag="cm_sb")
        nc.sync.dma_start(cm_sb[:], cm_d.ap().rearrange("j p q -> p j q"))

        wo_st = constp.tile([P, D], F32, name="wo_st", tag="wo_st")
        nc.sync.dma_start(wo_st[:], wo_d.ap())
        wor = constp.tile([P, D], F32R, name="wor", tag="wor")
        nc.vector.tensor_copy(wor[:], wo_st[:])

        prodr = tailp.tile([P, TOK_SLICE], F32R, name="prodr",
                           tag="prodr", bufs=1)

        tail_pr = {}

        def emit_tail_head(b):
            """Start consuming A2A(b): head product chain on Pool."""
            hp = hpp.tile([P, NC * TS], F16, name="hp", tag="hp")
            nc.gpsimd.dma_start(
                hp[:], a2a_out[b].rearrange("r p t -> p r t"))
            pr = tailp.tile([P, TS], F32, name="pr", tag="pr")
            nc.gpsimd.tensor_mul(pr[:], hp[:, 0:TS], hp[:, TS:2 * TS])
            for r in range(2, NC - 1):
                nc.gpsimd.tensor_mul(
                    pr[:], pr[:], hp[:, r * TS:(r + 1) * TS])
            tail_pr[b] = (pr, hp)

        def emit_tail_tail(b):
            """Finish A2A(b): final product multiply + out_proj slice."""
            pr, hp = tail_pr.pop(b)
            nc.vector.tensor_mul(
                prodr[:, b * TS:(b + 1) * TS], pr[:],
                hp[:, (NC - 1) * TS:NC * TS])
            for ttl in range(TS // P):
                tt = b * (TS // P) + ttl
                ysb = ysbp.tile([P, D], F32, name="ysb", tag="ysb")
                for nn in range(D // TQ):
                    accy = psA.tile([P, TQ], F32, name="accy",
                                    tag="mmacc")
                    nc.tensor.matmul(
                        accy[:],
                        prodr[:, tt * P:(tt + 1) * P],
                        wor[:, nn * TQ:(nn + 1) * TQ],
                        start=True, stop=True)
                    nc.vector.tensor_copy(
                        ysb[:, nn * TQ:(nn + 1) * TQ], accy[:])
                nc.sync.dma_start(y_d.ap()[tt * P:(tt + 1) * P, :],
                                  ysb[:])

        staged = {}
        pending = {}

        def make_load_steps(b, split_x=False):
            """Closures that DMA batch b's x and weights into SBUF (fp16)."""
            st = {"wr": {}}
            staged[b] = st

            def mk_x(kd, lo=0, hi=T):
                def f():
                    if "xr" not in st:
                        st["xr"] = xrp.tile([P, NKD * T], F16,
                                            name="xr", tag="xr")
                    eng = nc.sync if kd % 2 == 0 else nc.gpsimd
                    eng.dma_start(st["xr"][:, kd * T + lo: kd * T + hi],
                                  xT_d.ap()[b, kd * P:(kd + 1) * P, lo:hi])
                return f

            def mk_w(nm, wd):
                def f():
                    wt = wrp.tile([P, NKD * HD], F16, name=f"wr_{nm}",
                                  tag=f"wr_{nm}")
                    half = NKD // 2 * HD
                    rr = wd.ap()[b].rearrange("kd p m -> p kd m")
                    nc.sync.dma_start(wt[:, 0:half], rr[:, 0:NKD // 2])
                    nc.gpsimd.dma_start(wt[:, half:], rr[:, NKD // 2:])
                    st["wr"][nm] = wt
                return f

            wsteps = [mk_w(nm, wd)
                      for nm, wd in (("v", wv_d), ("q", wq_d),
                                     ("k", wk_d))]
            if split_x:
                # first batch: finer granularity so PE can start earlier
                xsteps = [mk_x(0, 0, TQ), mk_x(0, TQ, 2 * TQ),
                          mk_x(0, 2 * TQ, 3 * TQ), mk_x(0, 3 * TQ, T),
                          mk_x(1, 0, T // 2), mk_x(1, T // 2, T)]
                xsteps += [mk_x(kd) for kd in range(2, NKD)]
                steps = [wsteps[0]] + xsteps[:6] + [wsteps[1]] + \
                    xsteps[6:8] + [wsteps[2]] + xsteps[8:]
            else:
                xsteps = [mk_x(kd) for kd in range(NKD)]
                steps = [wsteps[0], xsteps[0], xsteps[1], wsteps[1],
                         xsteps[2], xsteps[3], wsteps[2]] + xsteps[4:]
            return steps

        def drain_pending(b, n=None):
            steps = pending.get(b, [])
            k = len(steps) if n is None else min(n, len(steps))
            for f in steps[:k]:
                f()
            pending[b] = steps[k:]

        def emit_projections(b, first):
            st = staged[b]
            xr = st["xr"]
            qkt = {}
            v_sb = None
            state = {"trans": 0}

            def emit_trans(dest, upto):
                while state["trans"] < upto:
                    tt = state["trans"]
                    vtp = psA.tile([P, P], F16, name="vtp", tag="mmacc")
                    nc.tensor.transpose(
                        vtp[:], dest[:, tt * P:(tt + 1) * P], eye_b[:])
                    nc.vector.tensor_copy(
                        v_sb[:, tt * HD:(tt + 1) * HD], vtp[:])
                    state["trans"] = tt + 1

            for nm in ("v", "q", "k"):
                wt = st["wr"][nm]
                dest = qkvp.tile([P, T], F16, name=f"{nm}T", tag=f"{nm}T")
                if b == 0 and first and nm == "v":
                    # kd-outer: start PE as soon as the first x tile
                    # lands; 4 chunk accumulators as halves of 2 pair
                    # tiles (4 PSUM banks)
                    pair0 = psS.tile([P, 2 * TQ], F32, name="pacc0",
                                     tag="accs")
                    pair1 = psS.tile([P, 2 * TQ], F32, name="pacc1",
                                     tag="accs")
                    accs4 = [pair0[:, 0:TQ], pair0[:, TQ:2 * TQ],
                             pair1[:, 0:TQ], pair1[:, TQ:2 * TQ]]
                    for kd in range(NKD):
                        for qc in range(NQC):
                            nc.tensor.matmul(
                                accs4[qc],
                                wt[:, kd * HD:(kd + 1) * HD],
                                xr[:, kd * T + qc * TQ: kd * T + (qc + 1) * TQ],
                                start=(kd == 0), stop=(kd == NKD - 1))
                    for qc in range(NQC):
                        nc.vector.tensor_copy(
                            dest[:, qc * TQ:(qc + 1) * TQ], accs4[qc])
                else:
                    for qc in range(NQC):
                        acc = psA.tile([P, TQ], F32, name="acc",
                                       tag="mmacc")
                        for kd in range(NKD):
                            nc.tensor.matmul(
                                acc[:],
                                wt[:, kd * HD:(kd + 1) * HD],
                                xr[:, kd * T + qc * TQ: kd * T + (qc + 1) * TQ],
                                start=(kd == 0), stop=(kd == NKD - 1))
                        nc.vector.tensor_copy(
                            dest[:, qc * TQ:(qc + 1) * TQ], acc[:])
                        if nm == "q":
                            # keep PE busy while DVE evicts: V transposes
                            emit_trans(qkt["v"], 4 * (qc + 1))
                qkt[nm] = dest
                if nm == "v":
                    v_sb = qkvp.tile([P, NTT * HD], F16, name="vS",
                                     tag="vS")
                    if b == 0 and first:
                        emit_trans(dest, NTT)
            emit_trans(qkt["v"], NTT)
            return qkt, v_sb

        for rep in range(nreps):
            first = rep == 0
            for b in range(B):
                if b == 0 and first:
                    pending[0] = make_load_steps(0, split_x=True)
                drain_pending(b)
                qkt, v_sb = emit_projections(b, first)

                # ---- causal attention, scoresT layout, pair tiles ----
                out_b = outbp.tile([P, T], F16, name="out_b", tag="out_b")
                for qc in range(NQC):
                    nkt = 4 * (qc + 1)
                    npair = nkt // 2
                    acco = psO.tile([P, TQ], F32, name="acco", tag="acco")
                    denb = psD.tile([P, TQ], F32, name="denb", tag="denb")
                    es = {}

                    def emit_scores(p, qc=qc, es=es):
                        sc = psS.tile([P, 2 * TQ], F32, name="sc",
                                      tag="accs")
                        for h in range(2):
                            kt = 2 * p + h
                            nc.tensor.matmul(
                                sc[:, h * TQ:(h + 1) * TQ],
                                qkt["k"][:, kt * P:(kt + 1) * P],
                                qkt["q"][:, qc * TQ:(qc + 1) * TQ],
                                start=True, stop=True)
                        e = eop.tile([P, 2 * TQ], F16, name="e", tag="e")
                        nc.scalar.activation(e[:], sc[:], Exp)
                        j = 2 * p - 4 * qc
                        if j >= 0:  # diagonal pair: zero invalid cols
                            nc.vector.tensor_mul(
                                e[:], e[:], cm_sb[:, j * TQ:(j + 2) * TQ])
                        es[p] = e

                    def emit_av(p, qc=qc, es=es, acco=acco, denb=denb,
                                nkt=nkt):
                        e = es.pop(p)
                        for h in range(2):
                            kt = 2 * p + h
                            nc.tensor.matmul(
                                acco[:],
                                v_sb[:, kt * HD:(kt + 1) * HD],
                                e[:, h * TQ:(h + 1) * TQ],
                                start=(kt == 0), stop=(kt == nkt - 1))
                        for h in range(2):
                            kt = 2 * p + h
                            nc.tensor.matmul(
                                denb[:], ones_b[:],
                                e[:, h * TQ:(h + 1) * TQ],
                                start=(kt == 0), stop=(kt == nkt - 1))

                    emit_scores(0)
                    for p in range(1, npair):
                        emit_scores(p)
                        emit_av(p - 1)
                    emit_av(npair - 1)

                    recb = scp.tile([P, TQ], F32, name="recb", tag="recb")
                    nc.vector.reciprocal_approx_fast(recb[:], denb[:])
                    nc.vector.tensor_mul(
                        out_b[:, qc * TQ:(qc + 1) * TQ], acco[:], recb[:])
                    # ship this qc's two core-slices into the A2A staging
                    for j in (2 * qc, 2 * qc + 1):
                        nc.sync.dma_start(
                            a2a_in[b][j],
                            out_b[:, j * TS:(j + 1) * TS])

                    if qc == 1 and not (b == 0 and first):
                        pb = b - 1 if b > 0 else B - 1
                        emit_tail_tail(pb)
                    if b + 1 < B:
                        if qc == 0:
                            pending[b + 1] = make_load_steps(b + 1)
                        drain_pending(b + 1, 4)
                    elif rep + 1 < nreps:
                        if qc == 0:
                            pending[0] = make_load_steps(0)
                        drain_pending(0, 4)

                nc.gpsimd.collective_compute(
                    "AllToAll", mybir.AluOpType.bypass,
                    replica_groups=rg,
                    ins=[a2a_in[b].opt()], outs=[a2a_out[b].opt()])
                emit_tail_head(b)

        emit_tail_tail(B - 1)

    nc.compile()
    _PROGRAM[nreps] = nc
    return nc


def make_in_maps(x, Wq, Wk, Wv, Wout, q_mask, k_mask, v_mask):
    x = np.asarray(x, np.float32)
    xT = np.ascontiguousarray(x.transpose(0, 2, 1)).astype(np.float16)

    wo = np.ascontiguousarray(np.asarray(Wout, np.float32).T)  # (HD, D)

    cm = np.zeros((4, P, TQ), np.float32)
    for j in range(4):
        for i in range(P):
            cm[j, i, j * P + i:] = 1.0
    cm = cm.astype(np.float16)
    eye = np.eye(P, dtype=np.float32).astype(np.float16)

    s = np.float32(1.0 / np.sqrt(HD))
    q_mask = np.asarray(q_mask, np.float32)
    k_mask = np.asarray(k_mask, np.float32)
    v_mask = np.asarray(v_mask, np.float32)
    Wq = np.asarray(Wq, np.float32)
    Wk = np.asarray(Wk, np.float32)
    Wv = np.asarray(Wv, np.float32)

    in_maps = []
    for c in range(NC):
        def pack(W, m, scale):
            out = np.empty((B, NKD, P, HD), np.float32)
            Wh = W[c * HD:(c + 1) * HD, :]                  # (HD, D)
            for b in range(B):
                Wp = (Wh * (m[b, c, 0, :, None] * scale)).T  # (D, HD)
                out[b] = Wp.reshape(NKD, P, HD)
            return out.astype(np.float16)
        in_maps.append({
            "xT": xT,
            "wq": pack(Wq, q_mask, s),
            "wk": pack(Wk, k_mask, np.float32(1.0)),
            "wv": pack(Wv, v_mask, np.float32(1.0)),
            "wo": wo,
            "cm": cm,
            "eye": eye,
        })
    return in_maps


def kernel(x, Wq, Wk, Wv, Wout, q_mask, k_mask, v_mask, mask=None):
    nc = build_program()
    in_maps = make_in_maps(x, Wq, Wk, Wv, Wout, q_mask, k_mask, v_mask)
    res = run_bass_kernel_spmd(nc, in_maps, core_ids=list(range(NC))).results
    # core c's y rows are ordered (b, local-token); its tokens are
    # [c*TS, (c+1)*TS) of every batch
    out = np.empty((B, T, D), np.float32)
    for c in range(NC):
        yc = res[c]["y"].reshape(B, TS, D)
        out[:, c * TS:(c + 1) * TS, :] = yc
    return out


# revision 10
# speedup vs baseline: 2544879.8048x; 1.2427x over previous
"""Bass/Trainium2 kernel for nn_BayesMultiheadAttention (B=4,T=2048,D=1024,H=8).

Sharding: tensor-parallel over heads. Core c computes head c (QKV proj +
causal attention) for all 4 batches; a per-batch AllToAll redistributes
per-head outputs into per-token-slice outputs (pipelined against the next
batch's compute); each core then does the multiplicative reduce over heads
and its slice of out_proj.

x and the QKV weights are fp16 on the host and DMA'd straight into SBUF
(no on-chip rounding passes); projection/attention matmuls run in fp16
(1 cycle/row at any free size), accumulating fp32 in PSUM. Scores PSUM,
softmax normalization, head product and out_proj stay fp32/f32r; the
AllToAll payload is fp16.

Attention runs on [128,1024] score PAIR tiles (2 PSUM banks each, 2 bufs
= 4 banks): one exp ACTIVATE covers two kt tiles, and emission is skewed
(scores of pair p+1 issue before AV/denb of pair p) so ACT latency hides
behind PE work. Softmax denominators accumulate in PSUM via per-tile
ones-matmuls. V transposes are interleaved into the Q projection chunks
to keep PE fed while DVE evicts. Dropout masks and the 1/sqrt(HD) scale
are folded into per-(core,batch) weight copies on the host. Softmax skips
max-subtraction (scores are small; exp cannot overflow).
"""
import numpy as np

import concourse.bacc as bacc
import concourse.mybir as mybir
import concourse.tile as tile
from concourse.bass_utils import run_bass_kernel_spmd

B, T, D, H = 4, 2048, 1024, 8
HD = 128          # head dim
P = 128           # partitions
NC = 8            # cores
TQ = 512          # qt chunk width
NKD = D // P      # 8 contraction tiles
NTT = T // P      # 16 token tiles per batch
NQC = T // TQ     # 4 qt chunks per batch
TS = T // NC      # 256: per-core token slice of one batch
TOK_SLICE = B * TS  # 1024 tokens per core in the tail

dt = mybir.dt
F32 = dt.float32
F32R = dt.float32r
F16 = dt.float16

_PROGRAM = {}


def build_program(nreps=1):
    global _PROGRAM
    if nreps in _PROGRAM:
        return _PROGRAM[nreps]
    nc = bacc.Bacc("TRN2", target_bir_lowering=False, debug=False,
                   num_devices=NC)

    xT_d = nc.dram_tensor("xT", [B, D, T], F16, kind="ExternalInput")
    wq_d = nc.dram_tensor("wq", [B, NKD, P, HD], F16, kind="ExternalInput")
    wk_d = nc.dram_tensor("wk", [B, NKD, P, HD], F16, kind="ExternalInput")
    wv_d = nc.dram_tensor("wv", [B, NKD, P, HD], F16, kind="ExternalInput")
    wo_d = nc.dram_tensor("wo", [HD, D], F32, kind="ExternalInput")
    cm_d = nc.dram_tensor("cm", [4, P, TQ], F16, kind="ExternalInput")
    eye_d = nc.dram_tensor("eye", [P, P], F16, kind="ExternalInput")
    y_d = nc.dram_tensor("y", [TOK_SLICE, D], F32, kind="ExternalOutput")

    rg = [list(range(NC))]
    Exp = mybir.ActivationFunctionType.Exp

    from contextlib import ExitStack
    with tile.TileContext(nc) as tc, ExitStack() as ctx:
        ec = ctx.enter_context
        constp = ec(tc.tile_pool(name="const", bufs=1))
        xrp = ec(tc.tile_pool(name="xr", bufs=1))
        wrp = ec(tc.tile_pool(name="wr", bufs=1))
        qkvp = ec(tc.tile_pool(name="qkv", bufs=1))
        eop = ec(tc.tile_pool(name="eo", bufs=4))
        scp = ec(tc.tile_pool(name="sc", bufs=3))
        outbp = ec(tc.tile_pool(name="outb", bufs=1))
        tailp = ec(tc.tile_pool(name="tail", bufs=2))
        hpp = ec(tc.tile_pool(name="hp", bufs=2))
        ysbp = ec(tc.tile_pool(name="ysb", bufs=2))
        psA = ec(tc.tile_pool(name="psA", bufs=2, space="PSUM"))
        psS = ec(tc.tile_pool(name="psS", bufs=2, space="PSUM"))
        psO = ec(tc.tile_pool(name="psO", bufs=1, space="PSUM"))
        psD = ec(tc.tile_pool(name="psD", bufs=1, space="PSUM"))
        dram = ec(tc.tile_pool(name="dram", bufs=1, space="DRAM"))
        a2a_in = [dram.tile([NC, P, TS], F16, name=f"a2a_in{b}",
                            tag=f"a2a_in{b}") for b in range(B)]
        a2a_out = [dram.tile([NC, P, TS], F16, name=f"a2a_out{b}",
                             tag=f"a2a_out{b}") for b in range(B)]

        ones_b = constp.tile([P, P], F16, name="ones_b", tag="ones_b")
        nc.vector.memset(ones_b[:], 1.0)

        eye_b = constp.tile([P, P], F16, name="eye_b", tag="eye_b")
        nc.sync.dma_start(eye_b[:], eye_d.ap())

        cm_sb = constp.tile([P, 4 * TQ], F16, name="cm_sb", tag="cm_sb")
        nc.gpsimd.dma_start(cm_sb[:], cm_d.ap().rearrange("j p q -> p j q"))

        wo_st = constp.tile([P, D], F32, name="wo_st", tag="wo_st")
        nc.gpsimd.dma_start(wo_st[:], wo_d.ap())
        wor = constp.tile([P, D], F32R, name="wor", tag="wor")
        nc.vector.tensor_copy(wor[:], wo_st[:])

        prodr = tailp.tile([P, TOK_SLICE], F32R, name="prodr",
                           tag="prodr", bufs=1)

        tail_pr = {}

        def emit_tail_head(b):
            """Start consuming A2A(b): head product chain on Pool."""
            hp = hpp.tile([P, NC * TS], F16, name="hp", tag="hp")
            nc.gpsimd.dma_start(
                hp[:], a2a_out[b].rearrange("r p t -> p r t"))
            pr = tailp.tile([P, TS], F32, name="pr", tag="pr")
            nc.gpsimd.tensor_mul(pr[:], hp[:, 0:TS], hp[:, TS:2 * TS])
            for r in range(2, NC - 1):
                nc.gpsimd.tensor_mul(
                    pr[:], pr[:], hp[:, r * TS:(r + 1) * TS])
            tail_pr[b] = (pr, hp)

        def emit_tail_tail(b):
            """Finish A2A(b): final product multiply + out_proj slice."""
            pr, hp = tail_pr.pop(b)
            nc.vector.tensor_mul(
                prodr[:, b * TS:(b + 1) * TS], pr[:],
                hp[:, (NC - 1) * TS:NC * TS])
            for ttl in range(TS // P):
                tt = b * (TS // P) + ttl
                ysb = ysbp.tile([P, D], F32, name="ysb", tag="ysb")
                for nn in range(D // TQ):
                    accy = psA.tile([P, TQ], F32, name="accy",
                                    tag="mmacc")
                    nc.tensor.matmul(
                        accy[:],
                        prodr[:, tt * P:(tt + 1) * P],
                        wor[:, nn * TQ:(nn + 1) * TQ],
                        start=True, stop=True)
                    nc.vector.tensor_copy(
                        ysb[:, nn * TQ:(nn + 1) * TQ], accy[:])
                nc.sync.dma_start(y_d.ap()[tt * P:(tt + 1) * P, :],
                                  ysb[:])

        staged = {}
        pending = {}

        def make_load_steps(b, split_x=False):
            """Closures that DMA batch b's x and weights into SBUF (fp16)."""
            st = {"wr": {}}
            staged[b] = st

            def mk_x(kd, lo=0, hi=T, ei=None):
                def f():
                    if "xr" not in st:
                        st["xr"] = xrp.tile([P, NKD * T], F16,
                                            name="xr", tag="xr")
                    e = ei if ei is not None else kd % 2
                    eng = nc.sync if e == 0 else nc.gpsimd
                    eng.dma_start(st["xr"][:, kd * T + lo: kd * T + hi],
                                  xT_d.ap()[b, kd * P:(kd + 1) * P, lo:hi])
                return f

            def mk_w(nm, wd):
                def f():
                    wt = wrp.tile([P, NKD * HD], F16, name=f"wr_{nm}",
                                  tag=f"wr_{nm}")
                    half = NKD // 2 * HD
                    rr = wd.ap()[b].rearrange("kd p m -> p kd m")
                    nc.sync.dma_start(wt[:, 0:half], rr[:, 0:NKD // 2])
                    nc.gpsimd.dma_start(wt[:, half:], rr[:, NKD // 2:])
                    st["wr"][nm] = wt
                return f

            wsteps = [mk_w(nm, wd)
                      for nm, wd in (("v", wv_d), ("q", wq_d),
                                     ("k", wk_d))]
            if split_x:
                # first batch: finer granularity so PE can start earlier,
                # quarters alternating across both DMA queues
                xsteps = [mk_x(0, 0, TQ, 0), mk_x(0, TQ, 2 * TQ, 1),
                          mk_x(0, 2 * TQ, 3 * TQ, 0), mk_x(0, 3 * TQ, T, 1),
                          mk_x(1, 0, T // 2, 0), mk_x(1, T // 2, T, 1)]
                xsteps += [mk_x(kd) for kd in range(2, NKD)]
                steps = [wsteps[0]] + xsteps[:6] + [wsteps[1]] + \
                    xsteps[6:8] + [wsteps[2]] + xsteps[8:]
            else:
                xsteps = [mk_x(kd) for kd in range(NKD)]
                steps = [wsteps[0], xsteps[0], xsteps[1], wsteps[1],
                         xsteps[2], xsteps[3], wsteps[2]] + xsteps[4:]
            return steps

        def drain_pending(b, n=None):
            steps = pending.get(b, [])
            k = len(steps) if n is None else min(n, len(steps))
            for f in steps[:k]:
                f()
            pending[b] = steps[k:]

        def emit_projections(b, first):
            st = staged[b]
            xr = st["xr"]
            qkt = {}
            v_sb = None
            state = {"trans": 0}

            def emit_trans(dest, upto):
                while state["trans"] < upto:
                    tt = state["trans"]
                    vtp = psA.tile([P, P], F16, name="vtp", tag="mmacc")
                    nc.tensor.transpose(
                        vtp[:], dest[:, tt * P:(tt + 1) * P], eye_b[:])
                    nc.vector.tensor_copy(
                        v_sb[:, tt * HD:(tt + 1) * HD], vtp[:])
                    state["trans"] = tt + 1

            for nm in ("v", "q", "k"):
                wt = st["wr"][nm]
                dest = qkvp.tile([P, T], F16, name=f"{nm}T", tag=f"{nm}T")
                if b == 0 and first and nm == "v":
                    # kd-outer: start PE as soon as the first x tile
                    # lands; 4 chunk accumulators as halves of 2 pair
                    # tiles (4 PSUM banks)
                    pair0 = psS.tile([P, 2 * TQ], F32, name="pacc0",
                                     tag="accs")
                    pair1 = psS.tile([P, 2 * TQ], F32, name="pacc1",
                                     tag="accs")
                    accs4 = [pair0[:, 0:TQ], pair0[:, TQ:2 * TQ],
                             pair1[:, 0:TQ], pair1[:, TQ:2 * TQ]]
                    for kd in range(NKD):
                        for qc in range(NQC):
                            nc.tensor.matmul(
                                accs4[qc],
                                wt[:, kd * HD:(kd + 1) * HD],
                                xr[:, kd * T + qc * TQ: kd * T + (qc + 1) * TQ],
                                start=(kd == 0), stop=(kd == NKD - 1))
                    for qc in range(NQC):
                        nc.vector.tensor_copy(
                            dest[:, qc * TQ:(qc + 1) * TQ], accs4[qc])
                else:
                    for qc in range(NQC):
                        acc = psA.tile([P, TQ], F32, name="acc",
                                       tag="mmacc")
                        for kd in range(NKD):
                            nc.tensor.matmul(
                                acc[:],
                                wt[:, kd * HD:(kd + 1) * HD],
                                xr[:, kd * T + qc * TQ: kd * T + (qc + 1) * TQ],
                                start=(kd == 0), stop=(kd == NKD - 1))
                        nc.vector.tensor_copy(
                            dest[:, qc * TQ:(qc + 1) * TQ], acc[:])
                        if nm == "q":
                            # keep PE busy while DVE evicts: V transposes
                            emit_trans(qkt["v"], 4 * (qc + 1))
                qkt[nm] = dest
                if nm == "v":
                    v_sb = qkvp.tile([P, NTT * HD], F16, name="vS",
                                     tag="vS")
                    if b == 0 and first:
                        emit_trans(dest, NTT)
            emit_trans(qkt["v"], NTT)
            return qkt, v_sb

        for rep in range(nreps):
            first = rep == 0
            for b in range(B):
                if b == 0 and first:
                    pending[0] = make_load_steps(0, split_x=True)
                drain_pending(b)
                qkt, v_sb = emit_projections(b, first)

                # ---- causal attention, scoresT layout, pair tiles ----
                # flat software pipeline over (qc, p): scores of element
                # i+1 issue before AV/denb of element i, across qc
                # boundaries, so PE never waits a full exp latency.
                out_b = outbp.tile([P, T], F16, name="out_b", tag="out_b")
                pairs = [(qc, p) for qc in range(NQC)
                         for p in range(2 * (qc + 1))]
                es = {}
                ps = {}

                def emit_scores(qp):
                    qc, p = qp
                    sc = psS.tile([P, 2 * TQ], F32, name="sc", tag="accs")
                    for h in range(2):
                        kt = 2 * p + h
                        nc.tensor.matmul(
                            sc[:, h * TQ:(h + 1) * TQ],
                            qkt["k"][:, kt * P:(kt + 1) * P],
                            qkt["q"][:, qc * TQ:(qc + 1) * TQ],
                            start=True, stop=True)
                    e = eop.tile([P, 2 * TQ], F16, name="e", tag="e")
                    nc.scalar.activation(e[:], sc[:], Exp)
                    j = 2 * p - 4 * qc
                    if j >= 0:  # diagonal pair: zero invalid cols
                        nc.vector.tensor_mul(
                            e[:], e[:], cm_sb[:, j * TQ:(j + 2) * TQ])
                    es[qp] = e

                def emit_av(qp):
                    qc, p = qp
                    nkt = 4 * (qc + 1)
                    npair = nkt // 2
                    if p == 0:
                        ps["acco"] = psO.tile([P, TQ], F32, name="acco",
                                              tag="acco")
                        ps["denb"] = psD.tile([P, TQ], F32, name="denb",
                                              tag="denb")
                    acco, denb = ps["acco"], ps["denb"]
                    e = es.pop(qp)
                    for h in range(2):
                        kt = 2 * p + h
                        nc.tensor.matmul(
                            acco[:],
                            v_sb[:, kt * HD:(kt + 1) * HD],
                            e[:, h * TQ:(h + 1) * TQ],
                            start=(kt == 0), stop=(kt == nkt - 1))
                    # denominator: DVE pair-sum, then one ones-matmul
                    esum = scp.tile([P, TQ], F16, name="esum", tag="esum")
                    nc.vector.tensor_add(esum[:], e[:, 0:TQ],
                                         e[:, TQ:2 * TQ])
                    nc.tensor.matmul(
                        denb[:], ones_b[:], esum[:],
                        start=(p == 0), stop=(p == npair - 1))
                    if p == npair - 1:
                        recb = scp.tile([P, TQ], F32, name="recb",
                                        tag="recb")
                        nc.vector.reciprocal_approx_fast(recb[:], denb[:])
                        nc.vector.tensor_mul(
                            out_b[:, qc * TQ:(qc + 1) * TQ], acco[:],
                            recb[:])
                        # ship this qc's two core-slices to A2A staging
                        for j in (2 * qc, 2 * qc + 1):
                            nc.sync.dma_start(
                                a2a_in[b][j],
                                out_b[:, j * TS:(j + 1) * TS])

                def qc_hooks(qc):
                    if qc == 1 and not (b == 0 and first):
                        pb = b - 1 if b > 0 else B - 1
                        emit_tail_tail(pb)
                    if b + 1 < B:
                        if qc == 0:
                            pending[b + 1] = make_load_steps(b + 1)
                        drain_pending(b + 1, 4)
                    elif rep + 1 < nreps:
                        if qc == 0:
                            pending[0] = make_load_steps(0)
                        drain_pending(0, 4)

                emit_scores(pairs[0])
                for i in range(1, len(pairs)):
                    emit_scores(pairs[i])
                    emit_av(pairs[i - 1])
                    if pairs[i][1] == 0:  # just crossed into qc=pairs[i][0]
                        qc_hooks(pairs[i][0] - 1)
                emit_av(pairs[-1])
                qc_hooks(NQC - 1)

                nc.gpsimd.collective_compute(
                    "AllToAll", mybir.AluOpType.bypass,
                    replica_groups=rg,
                    ins=[a2a_in[b].opt()], outs=[a2a_out[b].opt()])
                emit_tail_head(b)

        emit_tail_tail(B - 1)

    nc.compile()
    _PROGRAM[nreps] = nc
    return nc


def make_in_maps(x, Wq, Wk, Wv, Wout, q_mask, k_mask, v_mask):
    x = np.asarray(x, np.float32)
    xT = np.ascontiguousarray(x.transpose(0, 2, 1)).astype(np.float16)

    wo = np.ascontiguousarray(np.asarray(Wout, np.float32).T)  # (HD, D)

    cm = np.zeros((4, P, TQ), np.float32)
    for j in range(4):
        for i in range(P):
            cm[j, i, j * P + i:] = 1.0
    cm = cm.astype(np.float16)
    eye = np.eye(P, dtype=np.float32).astype(np.float16)

    s = np.float32(1.0 / np.sqrt(HD))
    q_mask = np.asarray(q_mask, np.float32)
    k_mask = np.asarray(k_mask, np.float32)
    v_mask = np.asarray(v_mask, np.float32)
    Wq = np.asarray(Wq, np.float32)
    Wk = np.asarray(Wk, np.float32)
    Wv = np.asarray(Wv, np.float32)

    in_maps = []
    for c in range(NC):
        def pack(W, m, scale):
            out = np.empty((B, NKD, P, HD), np.float32)
            Wh = W[c * HD:(c + 1) * HD, :]                  # (HD, D)
            for b in range(B):
                Wp = (Wh * (m[b, c, 0, :, None] * scale)).T  # (D, HD)
                out[b] = Wp.reshape(NKD, P, HD)
            return out.astype(np.float16)
        in_maps.append({
            "xT": xT,
            "wq": pack(Wq, q_mask, s),
            "wk": pack(Wk, k_mask, np.float32(1.0)),
            "wv": pack(Wv, v_mask, np.float32(1.0)),
            "wo": wo,
            "cm": cm,
            "eye": eye,
        })
    return in_maps


def kernel(x, Wq, Wk, Wv, Wout, q_mask, k_mask, v_mask, mask=None):
    nc = build_program()
    in_maps = make_in_maps(x, Wq, Wk, Wv, Wout, q_mask, k_mask, v_mask)
    res = run_bass_kernel_spmd(nc, in_maps, core_ids=list(range(NC))).results
    # core c's y rows are ordered (b, local-token); its tokens are
    # [c*TS, (c+1)*TS) of every batch
    out = np.empty((B, T, D), np.float32)
    for c in range(NC):
        yc = res[c]["y"].reshape(B, TS, D)
        out[:, c * TS:(c + 1) * TS, :] = yc
    return out


# revision 14
# speedup vs baseline: 1089315504.6164x; 428.0420x over previous
"""Bass/Trainium2 kernel for nn_BayesMultiheadAttention (B=4,T=2048,D=1024,H=8).

Sharding: tensor-parallel over heads. Core c computes head c (QKV proj +
causal attention) for all 4 batches; a per-batch AllToAll redistributes
per-head outputs into per-token-slice outputs (pipelined against the next
batch's compute); each core then does the multiplicative reduce over heads
and its slice of out_proj.

x and the QKV weights are fp16 on the host and DMA'd straight into SBUF
(no on-chip rounding passes); projection/attention matmuls run in fp16
(1 cycle/row at any free size), accumulating fp32 in PSUM. Scores PSUM,
softmax normalization, head product and out_proj stay fp32/f32r; the
AllToAll payload is fp16.

Attention runs on [128,1024] score PAIR tiles (2 PSUM banks each, 2 bufs
= 4 banks): one exp ACTIVATE covers two kt tiles, and emission is skewed
(scores of pair p+1 issue before AV/denb of pair p) so ACT latency hides
behind PE work. Softmax denominators accumulate in PSUM via per-tile
ones-matmuls. V transposes are interleaved into the Q projection chunks
to keep PE fed while DVE evicts. Dropout masks and the 1/sqrt(HD) scale
are folded into per-(core,batch) weight copies on the host. Softmax skips
max-subtraction (scores are small; exp cannot overflow).
"""
import numpy as np

import concourse.bacc as bacc
import concourse.mybir as mybir
import concourse.tile as tile
from concourse.bass_utils import run_bass_kernel_spmd

B, T, D, H = 4, 2048, 1024, 8
HD = 128          # head dim
P = 128           # partitions
NC = 8            # cores
TQ = 512          # qt chunk width
NKD = D // P      # 8 contraction tiles
NTT = T // P      # 16 token tiles per batch
NQC = T // TQ     # 4 qt chunks per batch
TS = T // NC      # 256: per-core token slice of one batch
TOK_SLICE = B * TS  # 1024 tokens per core in the tail

dt = mybir.dt
F32 = dt.float32
F32R = dt.float32r
F16 = dt.float16

_PROGRAM = {}


def build_program(nreps=1):
    global _PROGRAM
    if nreps in _PROGRAM:
        return _PROGRAM[nreps]
    nc = bacc.Bacc("TRN2", target_bir_lowering=False, debug=False,
                   num_devices=NC)

    xT_d = nc.dram_tensor("xT", [B, D, T], F16, kind="ExternalInput")
    wq_d = nc.dram_tensor("wq", [B, NKD, P, HD], F16, kind="ExternalInput")
    wk_d = nc.dram_tensor("wk", [B, NKD, P, HD], F16, kind="ExternalInput")
    wv_d = nc.dram_tensor("wv", [B, NKD, P, HD], F16, kind="ExternalInput")
    wo_d = nc.dram_tensor("wo", [HD, D], F32, kind="ExternalInput")
    cm_d = nc.dram_tensor("cm", [4, P, TQ], F16, kind="ExternalInput")
    eye_d = nc.dram_tensor("eye", [P, P], F16, kind="ExternalInput")
    y_d = nc.dram_tensor("y", [TOK_SLICE, D], F32, kind="ExternalOutput")

    rg = [list(range(NC))]
    Exp = mybir.ActivationFunctionType.Exp

    from contextlib import ExitStack
    with tile.TileContext(nc) as tc, ExitStack() as ctx:
        ec = ctx.enter_context
        constp = ec(tc.tile_pool(name="const", bufs=1))
        xrp = ec(tc.tile_pool(name="xr", bufs=1))
        wrp = ec(tc.tile_pool(name="wr", bufs=1))
        qkvp = ec(tc.tile_pool(name="qkv", bufs=1))
        eop = ec(tc.tile_pool(name="eo", bufs=4))
        scp = ec(tc.tile_pool(name="sc", bufs=3))
        outbp = ec(tc.tile_pool(name="outb", bufs=1))
        tailp = ec(tc.tile_pool(name="tail", bufs=2))
        hpp = ec(tc.tile_pool(name="hp", bufs=2))
        ysbp = ec(tc.tile_pool(name="ysb", bufs=2))
        psA = ec(tc.tile_pool(name="psA", bufs=2, space="PSUM"))
        psS = ec(tc.tile_pool(name="psS", bufs=2, space="PSUM"))
        psO = ec(tc.tile_pool(name="psO", bufs=1, space="PSUM"))
        psD = ec(tc.tile_pool(name="psD", bufs=1, space="PSUM"))
        dram = ec(tc.tile_pool(name="dram", bufs=1, space="DRAM"))
        a2a_in = [dram.tile([NC, P, TS], F16, name=f"a2a_in{b}",
                            tag=f"a2a_in{b}") for b in range(B)]
        a2a_out = [dram.tile([NC, P, TS], F16, name=f"a2a_out{b}",
                             tag=f"a2a_out{b}") for b in range(B)]

        ones_b = constp.tile([P, P], F16, name="ones_b", tag="ones_b")
        nc.vector.memset(ones_b[:], 1.0)

        eye_b = constp.tile([P, P], F16, name="eye_b", tag="eye_b")
        nc.sync.dma_start(eye_b[:], eye_d.ap())

        cm_sb = constp.tile([P, 4 * TQ], F16, name="cm_sb", tag="cm_sb")
        nc.gpsimd.dma_start(cm_sb[:], cm_d.ap().rearrange("j p q -> p j q"))

        wo_st = constp.tile([P, D], F32, name="wo_st", tag="wo_st")
        nc.gpsimd.dma_start(wo_st[:], wo_d.ap())
        wor = constp.tile([P, D], F32R, name="wor", tag="wor")
        nc.vector.tensor_copy(wor[:], wo_st[:])

        prodr = tailp.tile([P, TOK_SLICE], F32R, name="prodr",
                           tag="prodr", bufs=1)

        tail_pr = {}

        def emit_tail_head(b):
            """Start consuming A2A(b): head product chain on Pool."""
            hp = hpp.tile([P, NC * TS], F16, name="hp", tag="hp")
            nc.gpsimd.dma_start(
                hp[:], a2a_out[b].rearrange("r p t -> p r t"))
            pr = tailp.tile([P, TS], F32, name="pr", tag="pr")
            nc.gpsimd.tensor_mul(pr[:], hp[:, 0:TS], hp[:, TS:2 * TS])
            for r in range(2, NC - 1):
                nc.gpsimd.tensor_mul(
                    pr[:], pr[:], hp[:, r * TS:(r + 1) * TS])
            tail_pr[b] = (pr, hp)

        def emit_tail_tail(b):
            """Finish A2A(b): final product multiply + out_proj slice."""
            pr, hp = tail_pr.pop(b)
            nc.vector.tensor_mul(
                prodr[:, b * TS:(b + 1) * TS], pr[:],
                hp[:, (NC - 1) * TS:NC * TS])
            for ttl in range(TS // P):
                tt = b * (TS // P) + ttl
                ysb = ysbp.tile([P, D], F32, name="ysb", tag="ysb")
                for nn in range(D // TQ):
                    accy = psA.tile([P, TQ], F32, name="accy",
                                    tag="mmacc")
                    nc.tensor.matmul(
                        accy[:],
                        prodr[:, tt * P:(tt + 1) * P],
                        wor[:, nn * TQ:(nn + 1) * TQ],
                        start=True, stop=True)
                    nc.vector.tensor_copy(
                        ysb[:, nn * TQ:(nn + 1) * TQ], accy[:])
                nc.sync.dma_start(y_d.ap()[tt * P:(tt + 1) * P, :],
                                  ysb[:])

        staged = {}
        pending = {}

        def make_load_steps(b, split_x=False):
            """Closures that DMA batch b's x and weights into SBUF (fp16)."""
            st = {"wr": {}}
            staged[b] = st

            def mk_x(kd, lo=0, hi=T, ei=None):
                def f():
                    if "xr" not in st:
                        st["xr"] = xrp.tile([P, NKD * T], F16,
                                            name="xr", tag="xr")
                    e = ei if ei is not None else kd % 2
                    eng = nc.sync if e == 0 else nc.gpsimd
                    eng.dma_start(st["xr"][:, kd * T + lo: kd * T + hi],
                                  xT_d.ap()[b, kd * P:(kd + 1) * P, lo:hi])
                return f

            def mk_w(nm, wd):
                def f():
                    wt = wrp.tile([P, NKD * HD], F16, name=f"wr_{nm}",
                                  tag=f"wr_{nm}")
                    half = NKD // 2 * HD
                    rr = wd.ap()[b].rearrange("kd p m -> p kd m")
                    nc.sync.dma_start(wt[:, 0:half], rr[:, 0:NKD // 2])
                    nc.gpsimd.dma_start(wt[:, half:], rr[:, NKD // 2:])
                    st["wr"][nm] = wt
                return f

            wsteps = [mk_w(nm, wd)
                      for nm, wd in (("v", wv_d), ("q", wq_d),
                                     ("k", wk_d))]
            if split_x:
                # first batch: finer granularity so PE can start earlier,
                # quarters alternating across both DMA queues
                xsteps = [mk_x(0, 0, TQ, 0), mk_x(0, TQ, 2 * TQ, 1),
                          mk_x(0, 2 * TQ, 3 * TQ, 0), mk_x(0, 3 * TQ, T, 1),
                          mk_x(1, 0, T // 2, 0), mk_x(1, T // 2, T, 1)]
                xsteps += [mk_x(kd) for kd in range(2, NKD)]
                steps = [wsteps[0]] + xsteps[:6] + [wsteps[1]] + \
                    xsteps[6:8] + [wsteps[2]] + xsteps[8:]
            else:
                xsteps = [mk_x(kd) for kd in range(NKD)]
                steps = [wsteps[0], xsteps[0], xsteps[1], wsteps[1],
                         xsteps[2], xsteps[3], wsteps[2]] + xsteps[4:]
            return steps

        def drain_pending(b, n=None):
            steps = pending.get(b, [])
            k = len(steps) if n is None else min(n, len(steps))
            for f in steps[:k]:
                f()
            pending[b] = steps[k:]

        def emit_projections(b, first):
            st = staged[b]
            xr = st["xr"]
            qkt = {}
            v_sb = None
            state = {"trans": 0}

            def emit_trans(dest, upto):
                # transpose V tok-tiles in PAIRS into one PSUM tile so a
                # single 2x-mode DVE eviction covers both
                while state["trans"] < upto:
                    tt = state["trans"]
                    vtp = psA.tile([P, 2 * P], F16, name="vtp",
                                   tag="mmacc")
                    nc.tensor.transpose(
                        vtp[:, 0:P], dest[:, tt * P:(tt + 1) * P],
                        eye_b[:])
                    nc.tensor.transpose(
                        vtp[:, P:2 * P],
                        dest[:, (tt + 1) * P:(tt + 2) * P], eye_b[:])
                    nc.vector.tensor_copy(
                        v_sb[:, tt * HD:(tt + 2) * HD], vtp[:])
                    state["trans"] = tt + 2

            for nm in ("v", "q", "k"):
                wt = st["wr"][nm]
                dest = qkvp.tile([P, T], F16, name=f"{nm}T", tag=f"{nm}T")
                if b == 0 and first and nm == "v":
                    # kd-outer: start PE as soon as the first x tile
                    # lands; 4 chunk accumulators as halves of 2 pair
                    # tiles (4 PSUM banks)
                    pair0 = psS.tile([P, 2 * TQ], F32, name="pacc0",
                                     tag="accs")
                    pair1 = psS.tile([P, 2 * TQ], F32, name="pacc1",
                                     tag="accs")
                    accs4 = [pair0[:, 0:TQ], pair0[:, TQ:2 * TQ],
                             pair1[:, 0:TQ], pair1[:, TQ:2 * TQ]]
                    for kd in range(NKD):
                        for qc in range(NQC):
                            nc.tensor.matmul(
                                accs4[qc],
                                wt[:, kd * HD:(kd + 1) * HD],
                                xr[:, kd * T + qc * TQ: kd * T + (qc + 1) * TQ],
                                start=(kd == 0), stop=(kd == NKD - 1))
                    for qc in range(NQC):
                        nc.vector.tensor_copy(
                            dest[:, qc * TQ:(qc + 1) * TQ], accs4[qc])
                else:
                    for qc in range(NQC):
                        acc = psA.tile([P, TQ], F32, name="acc",
                                       tag="mmacc")
                        for kd in range(NKD):
                            nc.tensor.matmul(
                                acc[:],
                                wt[:, kd * HD:(kd + 1) * HD],
                                xr[:, kd * T + qc * TQ: kd * T + (qc + 1) * TQ],
                                start=(kd == 0), stop=(kd == NKD - 1))
                            if nm == "q" and kd in (3, 7):
                                # V transposes mid-chain: their evictions
                                # hide under the next 4 proj matmuls
                                emit_trans(qkt["v"], 4 * qc + 2 * (
                                    1 if kd == 3 else 2))
                        nc.vector.tensor_copy(
                            dest[:, qc * TQ:(qc + 1) * TQ], acc[:])
                qkt[nm] = dest
                if nm == "v":
                    v_sb = qkvp.tile([P, NTT * HD], F16, name="vS",
                                     tag="vS")
            emit_trans(qkt["v"], NTT)
            return qkt, v_sb

        for rep in range(nreps):
            first = rep == 0
            for b in range(B):
                if b == 0 and first:
                    pending[0] = make_load_steps(0, split_x=True)
                drain_pending(b)
                qkt, v_sb = emit_projections(b, first)

                # ---- causal attention, scoresT layout, pair tiles ----
                # flat software pipeline over (qc, p): scores of element
                # i+1 issue before AV/denb of element i, across qc
                # boundaries, so PE never waits a full exp latency.
                out_b = outbp.tile([P, T], F16, name="out_b", tag="out_b")
                pairs = [(qc, p) for qc in range(NQC)
                         for p in range(2 * (qc + 1))]
                es = {}
                ps = {}

                def emit_scores(qp):
                    qc, p = qp
                    sc = psS.tile([P, 2 * TQ], F32, name="sc", tag="accs")
                    for h in range(2):
                        kt = 2 * p + h
                        nc.tensor.matmul(
                            sc[:, h * TQ:(h + 1) * TQ],
                            qkt["k"][:, kt * P:(kt + 1) * P],
                            qkt["q"][:, qc * TQ:(qc + 1) * TQ],
                            start=True, stop=True)
                    e = eop.tile([P, 2 * TQ], F16, name="e", tag="e")
                    nc.scalar.activation(e[:], sc[:], Exp)
                    j = 2 * p - 4 * qc
                    if j >= 0:  # diagonal pair: zero invalid cols
                        nc.vector.tensor_mul(
                            e[:], e[:], cm_sb[:, j * TQ:(j + 2) * TQ])
                    es[qp] = e

                def emit_av(qp):
                    qc, p = qp
                    nkt = 4 * (qc + 1)
                    npair = nkt // 2
                    if p == 0:
                        ps["acco"] = psO.tile([P, TQ], F32, name="acco",
                                              tag="acco")
                        ps["denb"] = psD.tile([P, TQ], F32, name="denb",
                                              tag="denb")
                    acco, denb = ps["acco"], ps["denb"]
                    e = es.pop(qp)
                    for h in range(2):
                        kt = 2 * p + h
                        j = kt - 4 * qc
                        lo = max(j, 0) * P  # cols < lo are masked zeros
                        nc.tensor.matmul(
                            acco[:, lo:TQ],
                            v_sb[:, kt * HD:(kt + 1) * HD],
                            e[:, h * TQ + lo:(h + 1) * TQ],
                            start=(kt == 0), stop=(kt == nkt - 1))
                    # denominator: DVE pair-sum, then one ones-matmul
                    j0 = 2 * p - 4 * qc
                    lo = max(j0, 0) * P
                    esum = scp.tile([P, TQ], F16, name="esum", tag="esum")
                    nc.vector.tensor_add(esum[:, lo:TQ],
                                         e[:, lo:TQ],
                                         e[:, TQ + lo:2 * TQ])
                    nc.tensor.matmul(
                        denb[:, lo:TQ], ones_b[:], esum[:, lo:TQ],
                        start=(p == 0), stop=(p == npair - 1))
                    if p == npair - 1:
                        recb = scp.tile([P, TQ], F32, name="recb",
                                        tag="recb")
                        nc.vector.reciprocal_approx_fast(recb[:], denb[:])
                        nc.vector.tensor_mul(
                            out_b[:, qc * TQ:(qc + 1) * TQ], acco[:],
                            recb[:])
                        # ship this qc's two core-slices to A2A staging
                        for j in (2 * qc, 2 * qc + 1):
                            nc.sync.dma_start(
                                a2a_in[b][j],
                                out_b[:, j * TS:(j + 1) * TS])

                def qc_hooks(qc):
                    if qc == 2 and not (b == 0 and first):
                        pb = b - 1 if b > 0 else B - 1
                        emit_tail_tail(pb)
                    if b + 1 < B:
                        if qc == 0:
                            pending[b + 1] = make_load_steps(b + 1)
                        drain_pending(b + 1, 4)
                    elif rep + 1 < nreps:
                        if qc == 0:
                            pending[0] = make_load_steps(0)
                        drain_pending(0, 4)

                emit_scores(pairs[0])
                for i in range(1, len(pairs)):
                    emit_scores(pairs[i])
                    emit_av(pairs[i - 1])
                    if pairs[i][1] == 0:  # just crossed into qc=pairs[i][0]
                        qc_hooks(pairs[i][0] - 1)
                emit_av(pairs[-1])
                qc_hooks(NQC - 1)

                nc.gpsimd.collective_compute(
                    "AllToAll", mybir.AluOpType.bypass,
                    replica_groups=rg,
                    ins=[a2a_in[b].opt()], outs=[a2a_out[b].opt()])
                emit_tail_head(b)

        emit_tail_tail(B - 1)

    nc.compile()
    _PROGRAM[nreps] = nc
    return nc


def make_in_maps(x, Wq, Wk, Wv, Wout, q_mask, k_mask, v_mask):
    x = np.asarray(x, np.float32)
    xT = np.ascontiguousarray(x.transpose(0, 2, 1)).astype(np.float16)

    wo = np.ascontiguousarray(np.asarray(Wout, np.float32).T)  # (HD, D)

    cm = np.zeros((4, P, TQ), np.float32)
    for j in range(4):
        for i in range(P):
            cm[j, i, j * P + i:] = 1.0
    cm = cm.astype(np.float16)
    eye = np.eye(P, dtype=np.float32).astype(np.float16)

    s = np.float32(1.0 / np.sqrt(HD))
    q_mask = np.asarray(q_mask, np.float32)
    k_mask = np.asarray(k_mask, np.float32)
    v_mask = np.asarray(v_mask, np.float32)
    Wq = np.asarray(Wq, np.float32)
    Wk = np.asarray(Wk, np.float32)
    Wv = np.asarray(Wv, np.float32)

    in_maps = []
    for c in range(NC):
        def pack(W, m, scale):
            out = np.empty((B, NKD, P, HD), np.float32)
            Wh = W[c * HD:(c + 1) * HD, :]                  # (HD, D)
            for b in range(B):
                Wp = (Wh * (m[b, c, 0, :, None] * scale)).T  # (D, HD)
                out[b] = Wp.reshape(NKD, P, HD)
            return out.astype(np.float16)
        in_maps.append({
            "xT": xT,
            "wq": pack(Wq, q_mask, s),
            "wk": pack(Wk, k_mask, np.float32(1.0)),
            "wv": pack(Wv, v_mask, np.float32(1.0)),
            "wo": wo,
            "cm": cm,
            "eye": eye,
        })
    return in_maps


def kernel(x, Wq, Wk, Wv, Wout, q_mask, k_mask, v_mask, mask=None):
    nc = build_program()
    in_maps = make_in_maps(x, Wq, Wk, Wv, Wout, q_mask, k_mask, v_mask)
    res = run_bass_kernel_spmd(nc, in_maps, core_ids=list(range(NC))).results
    # core c's y rows are ordered (b, local-token); its tokens are
    # [c*TS, (c+1)*TS) of every batch
    out = np.empty((B, T, D), np.float32)
    for c in range(NC):
        yc = res[c]["y"].reshape(B, TS, D)
        out[:, c * TS:(c + 1) * TS, :] = yc
    return out
